# revision 20
# baseline (speedup 1.0000x reference)
"""Multi-head attention (RoPE, causal) Trainium2 Bass kernel, 8-way sharded.

Sharding: core c handles batch b = c//2 and head-group hg = c%2 (8 of 16
heads). Each core computes Q/K projections for its head slice in transposed
layout (QT/KT: [hd, l] with de-interleaved RoPE pairs), V projection in
natural layout, runs causal attention per head with scores kept transposed
(S^T[k, q], keys on partitions), and a partial output projection
out^T = Wo_slice @ attn^T. Host sums the two head-group partials per batch,
transposes back, and adds the effective output bias (bo + Wo @ bv; the V
bias is folded out of the kernel analytically).

Schedule: projections are split into column halves (V/K/Q for l in
[0,1024) first) so the attention blocks for queries < 1024 — whose exp
stream on the ACT engine is the kernel's hard floor — start while the
second-half projections still run on the PE. Attention keeps two key
tiles per [128,1024] PSUM score tile so exp runs as wide ACT
instructions; the softmax denominator rides as a ones-column through the
PV matmul; per-block normalization is reciprocal_approx_fast (DVE) +
GpSimd partition broadcast + DVE multiply. The output projection is
interleaved per 512-column chunk into the ACT-bound attention window.
"""

from contextlib import ExitStack

import ml_dtypes
import numpy as np

import concourse.bass as bass
import concourse.mybir as mybir
import concourse.tile as tile
from concourse import bacc
from concourse.bass_utils import run_bass_kernel_spmd

F32 = mybir.dt.float32
BF16 = mybir.dt.bfloat16
AF = mybir.ActivationFunctionType

B, L, D = 4, 2048, 1024
H, HD = 16, 64          # global heads, head dim
HPC = 8                 # heads per core
DH = HPC * HD           # 512: per-core projected width
KT = L // 128           # 16 key tiles
NCORES = 8
ROPE_BASE = 10000.0

_cache: dict = {}


def _build(compile=True):
    if "nc" in _cache:
        return _cache["nc"]

    nc = bacc.Bacc("TRN2", target_bir_lowering=False, debug=False)

    qTd = nc.dram_tensor("qT", [D, L], BF16, kind="ExternalInput").ap()
    kTd = nc.dram_tensor("kT", [D, L], BF16, kind="ExternalInput").ap()
    vTd = nc.dram_tensor("vT", [D, L], BF16, kind="ExternalInput").ap()
    wqd = nc.dram_tensor("wq", [128, 8 * DH], BF16, kind="ExternalInput").ap()
    wkd = nc.dram_tensor("wk", [128, 8 * DH], BF16, kind="ExternalInput").ap()
    wvd = nc.dram_tensor("wv", [128, 8 * DH], BF16, kind="ExternalInput").ap()
    wod = nc.dram_tensor("wo", [128, 4 * D], BF16, kind="ExternalInput").ap()
    bqc = nc.dram_tensor("bqc", [128, 4], F32, kind="ExternalInput").ap()
    bkc = nc.dram_tensor("bkc", [128, 4], F32, kind="ExternalInput").ap()
    cosP = nc.dram_tensor("cosP", [128, L], BF16, kind="ExternalInput").ap()
    sinP = nc.dram_tensor("sinP", [128, L], BF16, kind="ExternalInput").ap()
    maskc = nc.dram_tensor("maskc", [128, 128], BF16, kind="ExternalInput").ap()
    outT = nc.dram_tensor("outT", [D, L], F32, kind="ExternalOutput").ap()

    with tile.TileContext(nc) as tc, ExitStack() as ctx:
        const = ctx.enter_context(tc.tile_pool(name="const", bufs=1))
        pers = ctx.enter_context(tc.tile_pool(name="pers", bufs=1))
        px = ctx.enter_context(tc.tile_pool(name="px", bufs=3))
        ptw = ctx.enter_context(tc.tile_pool(name="ptw", bufs=4))
        psw = ctx.enter_context(tc.tile_pool(name="psw", bufs=2))
        ppt = ctx.enter_context(tc.tile_pool(name="ppt", bufs=4))
        pnm = ctx.enter_context(tc.tile_pool(name="pnm", bufs=3))
        pout = ctx.enter_context(tc.tile_pool(name="pout", bufs=4))
        pacc = ctx.enter_context(tc.tile_pool(name="pacc", bufs=2, space="PSUM"))
        psS = ctx.enter_context(tc.tile_pool(name="psS", bufs=2, space="PSUM"))
        psO = ctx.enter_context(tc.tile_pool(name="psO", bufs=2, space="PSUM"))

        # Pin the activation table to the one set that serves every ACT
        # function used here (Exp, Ln, Identity, Copy) so the table-load
        # pass never needs to thrash between per-function default tables.
        from concourse.hw_specs import get_activation_tables
        _tabs = list(get_activation_tables(nc.m.arch).items())
        _need = {AF.Exp, AF.Ln, AF.Identity, AF.Copy}
        _tid = next(i for i, (_n, s) in enumerate(_tabs) if _need <= s)
        nc.scalar.add_instruction(mybir.InstLoadActFuncSet(
            name=nc.scalar.bass.get_next_instruction_name(),
            ins=[], outs=[], act_func_set_id=_tid))

        # ---------------- constants (loads staggered in the schedule) --
        wv_t = const.tile([128, 8 * DH], BF16, tag="wv")
        wk_t = const.tile([128, 8 * DH], BF16, tag="wk")
        wq_t = const.tile([128, 8 * DH], BF16, tag="wq")
        wo_t = const.tile([128, 4 * D], BF16, tag="wo")
        cos_t = const.tile([128, L], BF16, tag="cos")
        sin_t = const.tile([128, L], BF16, tag="sin")
        mask_t = const.tile([128, 128], BF16, tag="mask")
        bq_t = const.tile([128, 4], F32, tag="bq")
        bk_t = const.tile([128, 4], F32, tag="bk")
        nc.sync.dma_start(wv_t[:], wvd[:])

        kt_m = [pers.tile([128, L], BF16, tag=f"kt{m}", name=f"kt{m}")
                for m in range(4)]
        qt_m = [pers.tile([128, L], BF16, tag=f"qt{m}", name=f"qt{m}")
                for m in range(4)]
        ot_m = [pers.tile([128, L], BF16, tag=f"ot{m}", name=f"ot{m}")
                for m in range(4)]
        va = [pers.tile([128, HPC * 65], BF16, tag=f"va{t}", name=f"va{t}")
              for t in range(KT)]
        for t in range(KT):
            ones_view = va[t].rearrange("p (h x) -> p h x", x=65)[:, :, 64:65]
            nc.gpsimd.memset(ones_view, 1.0)

        # ---------------- projection emitters -------------------------
        vx_tiles = {}

        def emit_v_group(g, li_lo=0, li_hi=8, on_act=True):
            """V projection for l-tiles [li_lo, li_hi) of half g."""
            if li_lo == 0:
                vp = []
                for kk in range(8):
                    x = px.tile([128, 1024], BF16, tag=f"x{kk}",
                                name=f"vx{kk}")
                    nc.sync.dma_start(
                        x[:], vTd[kk * 128:(kk + 1) * 128,
                                  g * 1024:(g + 1) * 1024])
                    vp.append(x)
                vx_tiles[g] = vp
            vp = vx_tiles[g]
            for li in range(li_lo, li_hi):
                lt = g * 8 + li
                ps = pacc.tile([128, DH], F32, tag="acc", name=f"pv{li}")
                for kk in range(8):
                    nc.tensor.matmul(
                        ps[:],
                        vp[kk][:, li * 128:(li + 1) * 128],
                        wv_t[:, kk * DH:(kk + 1) * DH],
                        start=(kk == 0), stop=(kk == 7),
                    )
                out_view = va[lt].rearrange(
                    "p (h x) -> p h x", x=65)[:, :, 0:64]
                if on_act:
                    nc.scalar.copy(out_view, ps[:])
                else:
                    nc.vector.tensor_copy(out_view, ps[:])

        qk_x_tiles = {}

        def emit_qk_group(xT, w_t, bias_t, dst, g, mi_lo=0, mi_hi=4,
                          on_act=True, key=None):
            """Q/K projection + RoPE, m-tiles [mi_lo, mi_hi) of half g.
            mi-outer order: each m-tile is drained and RoPE'd as soon as its
            two 512-column chunks finish, so dependent attention blocks can
            start before the rest of the projection."""
            c0, c1 = g * 1024, (g + 1) * 1024
            if mi_lo == 0:
                xp = []
                for kk in range(8):
                    x = px.tile([128, 1024], BF16, tag=f"x{kk}", name=f"x{kk}")
                    nc.sync.dma_start(x[:], xT[kk * 128:(kk + 1) * 128, c0:c1])
                    xp.append(x)
                qk_x_tiles[key] = xp
            xp = qk_x_tiles[key]
            for mi in range(mi_lo, mi_hi):
                traw = ptw.tile([128, 1024], BF16, tag="traw",
                                name=f"traw{mi}")
                for njl in range(2):
                    ps = pacc.tile([128, 512], F32, tag="acc", name=f"pp{mi}")
                    for kk in range(8):
                        nc.tensor.matmul(
                            ps[:],
                            w_t[:, kk * DH + mi * 128:kk * DH + (mi + 1) * 128],
                            xp[kk][:, njl * 512:(njl + 1) * 512],
                            start=(kk == 0), stop=(kk == 7),
                        )
                    dslice = traw[:, njl * 512:(njl + 1) * 512]
                    if on_act:
                        nc.scalar.activation(dslice, ps[:], AF.Identity,
                                             bias=bias_t[:, mi:mi + 1])
                    else:
                        nc.vector.tensor_scalar_add(dslice, ps[:],
                                                    bias_t[:, mi:mi + 1])
                sw = psw.tile([128, 1024], BF16, tag="sw", name=f"sw{mi}")
                for blk in range(4):
                    srcb = blk ^ 1
                    nc.gpsimd.dma_start(
                        sw[blk * 32:(blk + 1) * 32, :],
                        traw[srcb * 32:(srcb + 1) * 32, :])
                nc.vector.tensor_mul(sw[:], sw[:], sin_t[:, c0:c1])
                nc.vector.tensor_mul(traw[:], traw[:], cos_t[:, c0:c1])
                nc.vector.tensor_add(dst[mi][:, c0:c1], traw[:], sw[:])

        # ---------------- attention + output emitters -----------------
        def emit_b_block(b, h_lo=0, h_hi=HPC):
            q0 = b * 512
            for h in range(h_lo, h_hi):
                mi, pb = h // 2, (h % 2) * 64
                o_ps = psO.tile([128, 512], F32, tag="o", name=f"o{h}")
                nkt = 4 * (b + 1)
                for p in range(nkt // 2):
                    kt0, kt1 = 2 * p, 2 * p + 1
                    s_ps = psS.tile([128, 1024], F32, tag="s", name=f"s{p}")
                    pt = ppt.tile([128, 1024], BF16, tag="pt", name=f"pt{p}")
                    subs = []
                    for i, kt in enumerate((kt0, kt1)):
                        sub = max(0, kt * 128 - q0)
                        subs.append(sub)
                        nc.tensor.matmul(
                            s_ps[:, i * 512 + sub:(i + 1) * 512],
                            kt_m[mi][pb:pb + 64, kt * 128:(kt + 1) * 128],
                            qt_m[mi][pb:pb + 64, q0 + sub:q0 + 512],
                            start=True, stop=True,
                        )
                    if subs[1] == 0:
                        nc.scalar.activation(pt[:, 0:1024], s_ps[:, 0:1024],
                                             AF.Exp, scale=0.125)
                    else:
                        # diagonal pair: exp only the computed ranges
                        nc.scalar.activation(
                            pt[:, subs[0]:512], s_ps[:, subs[0]:512],
                            AF.Exp, scale=0.125)
                        nc.scalar.activation(
                            pt[:, 512 + subs[1]:1024],
                            s_ps[:, 512 + subs[1]:1024],
                            AF.Exp, scale=0.125)
                    for i, kt in enumerate((kt0, kt1)):
                        sub = subs[i]
                        diag = kt >= 4 * b
                        lhs = va[kt][:, h * 65:h * 65 + 65]
                        if diag:
                            nc.vector.tensor_mul(
                                pt[:, i * 512 + sub:i * 512 + sub + 128],
                                pt[:, i * 512 + sub:i * 512 + sub + 128],
                                mask_t[:])
                        nc.tensor.matmul(
                            o_ps[0:65, sub:512], lhs,
                            pt[:, i * 512 + sub:(i + 1) * 512],
                            start=(kt == 0), stop=(kt == nkt - 1),
                            skip_group_check=True,
                        )
                # normalize block: ot = o * (1/colsum). The reciprocal is
                # exp(-ln(colsum)) on the ACT engine (Ln and Exp share one
                # activation table), avoiding both the DVE's serial per-lane
                # reciprocal and any cross-partition DMA bounce; o and the
                # colsum row are copied out of PSUM first so the accumulator
                # bank recycles immediately.
                o_sb = pnm.tile([65, 512], F32, tag="osb", name=f"on{h}")
                nc.vector.tensor_copy(o_sb[:], o_ps[0:65, :])
                rs2 = pnm.tile([1, 512], F32, tag="rs2", name=f"rs2{h}")
                lncs = pnm.tile([1, 512], F32, tag="lncs", name=f"ln{h}")
                nc.scalar.activation(lncs[:], o_sb[64:65, :], AF.Ln)
                nc.scalar.activation(rs2[:], lncs[:], AF.Exp, scale=-1.0)
                rcb = pnm.tile([64, 512], F32, tag="rcb", name=f"rcb{h}")
                nc.gpsimd.partition_broadcast(rcb[:], rs2[:], channels=64)
                nc.vector.tensor_mul(
                    ot_m[mi][pb:pb + 64, q0:q0 + 512], o_sb[0:64, :], rcb[:])

        def emit_c(nj):
            for mo in range(8):
                ps = pacc.tile([128, 512], F32, tag="acc", name=f"c{mo}")
                for kti in range(4):
                    nc.tensor.matmul(
                        ps[:],
                        wo_t[:, kti * D + mo * 128:kti * D + (mo + 1) * 128],
                        ot_m[kti][:, nj * 512:(nj + 1) * 512],
                        start=(kti == 0), stop=(kti == 3),
                    )
                osb = pout.tile([128, 512], F32, tag="osb", name=f"osb{mo}")
                nc.vector.tensor_copy(osb[:], ps[:])
                # split over partition halves -> two DMA queues in parallel
                eng = nc.sync if nj == 3 else nc.gpsimd
                for ph in range(2):
                    eng.dma_start(
                        outT[mo * 128 + ph * 64:mo * 128 + (ph + 1) * 64,
                             nj * 512:(nj + 1) * 512],
                        osb[ph * 64:(ph + 1) * 64, :])

        # ---------------- schedule ------------------------------------
        # Fine-grained interleave: attention head-pairs start as soon as
        # their K/Q m-tile and V l-tiles exist; second-half projections
        # backfill the PE while the ACT engine chews block exps.
        emit_v_group(0, 0, 4, on_act=True)
        nc.sync.dma_start(wk_t[:], wkd[:])
        nc.sync.dma_start(bk_t[:], bkc[:])
        nc.sync.dma_start(bq_t[:], bqc[:])
        nc.sync.dma_start(cos_t[:], cosP[:])
        nc.sync.dma_start(sin_t[:], sinP[:])
        nc.sync.dma_start(mask_t[:], maskc[:])
        emit_qk_group(kTd, wk_t, bk_t, kt_m, 0, 0, 1, on_act=True, key="k0")
        nc.sync.dma_start(wq_t[:], wqd[:])
        emit_qk_group(qTd, wq_t, bq_t, qt_m, 0, 0, 1, on_act=True, key="q0")
        nc.sync.dma_start(wo_t[:], wod[:])
        emit_b_block(0, 0, 2)
        emit_v_group(0, 4, 8, on_act=True)
        emit_qk_group(kTd, wk_t, bk_t, kt_m, 0, 1, 2, on_act=True, key="k0")
        emit_qk_group(qTd, wq_t, bq_t, qt_m, 0, 1, 2, on_act=True, key="q0")
        emit_b_block(0, 2, 4)
        emit_qk_group(kTd, wk_t, bk_t, kt_m, 0, 2, 3, on_act=True, key="k0")
        emit_qk_group(qTd, wq_t, bq_t, qt_m, 0, 2, 3, on_act=True, key="q0")
        emit_b_block(0, 4, 6)
        emit_qk_group(kTd, wk_t, bk_t, kt_m, 0, 3, 4, on_act=True, key="k0")
        emit_qk_group(qTd, wq_t, bq_t, qt_m, 0, 3, 4, on_act=True, key="q0")
        emit_b_block(0, 6, 8)
        emit_b_block(1, 0, 2)
        emit_v_group(1, 0, 4, on_act=False)
        emit_b_block(1, 2, 4)
        emit_v_group(1, 4, 8, on_act=False)
        emit_b_block(1, 4, 6)
        emit_qk_group(kTd, wk_t, bk_t, kt_m, 1, 0, 2, on_act=False, key="k1")
        emit_b_block(1, 6, 8)
        emit_qk_group(kTd, wk_t, bk_t, kt_m, 1, 2, 4, on_act=False, key="k1")
        emit_qk_group(qTd, wq_t, bq_t, qt_m, 1, 0, 4, on_act=False, key="q1")
        emit_b_block(2)
        emit_c(0)
        emit_c(1)
        emit_b_block(3, 0, 4)
        emit_c(2)
        emit_b_block(3, 4, 8)
        emit_c(3)

    if compile:
        nc.compile()
        _cache["nc"] = nc
    return nc


def _prep(q, k, v, Wq, bq, Wk, bk, Wv, bv, Wo, bo):
    """Build the 8 per-core input maps (host-side shard + layout prep)."""
    bf16 = ml_dtypes.bfloat16
    # de-interleave permutation within each head: evens then odds
    perm = np.concatenate([np.arange(0, HD, 2), np.arange(1, HD, 2)])

    # RoPE tables in de-interleaved layout, tiled x2 over partitions
    inv_freq = 1.0 / (ROPE_BASE ** (np.arange(0, HD // 2, dtype=np.float64)
                                    * 2.0 / HD))
    t = np.arange(L, dtype=np.float64)
    freqs = inv_freq[:, None] * t[None, :]            # [32, L]
    cos64 = np.cos(np.concatenate([freqs, freqs], axis=0))   # [64, L]
    sin64 = np.sin(np.concatenate([freqs, freqs], axis=0))
    sin64[:32] *= -1.0
    cosP = np.tile(cos64, (2, 1)).astype(bf16)        # [128, L]
    sinP = np.tile(sin64, (2, 1)).astype(bf16)

    # causal mask in S^T space: keep k <= q
    kk, qq = np.meshgrid(np.arange(128), np.arange(128), indexing="ij")
    mask = (kk <= qq).astype(bf16)

    qTb = [np.ascontiguousarray(q[b_i].T.astype(bf16)) for b_i in range(B)]
    kTb = [np.ascontiguousarray(k[b_i].T.astype(bf16)) for b_i in range(B)]
    vTb = [np.ascontiguousarray(v[b_i].T.astype(bf16)) for b_i in range(B)]

    def wtile(WT, nk):
        # [nk*128, F] -> [128, nk*F] with kk-slices side by side
        F = WT.shape[1]
        return np.ascontiguousarray(
            WT.reshape(nk, 128, F).transpose(1, 0, 2).reshape(128, nk * F)
            .astype(bf16))

    in_maps = []
    for c in range(NCORES):
        b_i, hg = c // 2, c % 2
        rows = hg * DH + (np.arange(DH).reshape(HPC, HD)[:, perm]).reshape(-1)
        in_maps.append({
            "qT": qTb[b_i],
            "kT": kTb[b_i],
            "vT": vTb[b_i],
            "wq": wtile(Wq[rows, :].T, 8),
            "wk": wtile(Wk[rows, :].T, 8),
            "wv": wtile(Wv[hg * DH:(hg + 1) * DH, :].T, 8),
            "wo": wtile(Wo[:, hg * DH:(hg + 1) * DH].T, 4),
            "bqc": np.ascontiguousarray(bq[rows].reshape(4, 128).T),
            "bkc": np.ascontiguousarray(bk[rows].reshape(4, 128).T),
            "cosP": cosP,
            "sinP": sinP,
            "maskc": mask,
        })
    return in_maps


def _assemble(results, bo, Wo, bv):
    bo_eff = (bo + Wo.astype(np.float64) @ bv.astype(np.float64)).astype(
        np.float32)
    out = np.empty((B, L, D), dtype=np.float32)
    for b_i in range(B):
        acc = results[2 * b_i]["outT"] + results[2 * b_i + 1]["outT"]
        out[b_i] = acc.T + bo_eff[None, :]
    return out


def kernel(q, k, v, Wq, bq, Wk, bk, Wv, bv, Wo, bo):
    q = np.asarray(q, dtype=np.float32)
    k = np.asarray(k, dtype=np.float32)
    v = np.asarray(v, dtype=np.float32)
    Wq = np.asarray(Wq, dtype=np.float32)
    Wk = np.asarray(Wk, dtype=np.float32)
    Wv = np.asarray(Wv, dtype=np.float32)
    Wo = np.asarray(Wo, dtype=np.float32)
    bq = np.asarray(bq, dtype=np.float32)
    bk = np.asarray(bk, dtype=np.float32)
    bv = np.asarray(bv, dtype=np.float32)
    bo = np.asarray(bo, dtype=np.float32)

    nc = _build()
    in_maps = _prep(q, k, v, Wq, bq, Wk, bk, Wv, bv, Wo, bo)
    res = run_bass_kernel_spmd(nc, in_maps, core_ids=list(range(NCORES)))
    return _assemble(res.results, bo, Wo, bv)


# revision 21
# speedup vs baseline: 1.0090x; 1.0090x over previous
"""Multi-head attention (RoPE, causal) Trainium2 Bass kernel, 8-way sharded.

Sharding: core c handles batch b = c//2 and head-group hg = c%2 (8 of 16
heads). Each core computes Q/K projections for its head slice in transposed
layout (QT/KT: [hd, l] with de-interleaved RoPE pairs), V projection in
natural layout, runs causal attention per head with scores kept transposed
(S^T[k, q], keys on partitions), and a partial output projection
out^T = Wo_slice @ attn^T. Host sums the two head-group partials per batch,
transposes back, and adds the effective output bias (bo + Wo @ bv; the V
bias is folded out of the kernel analytically).

Schedule: projections are split into column halves (V/K/Q for l in
[0,1024) first) so the attention blocks for queries < 1024 — whose exp
stream on the ACT engine is the kernel's hard floor — start while the
second-half projections still run on the PE. Attention keeps two key
tiles per [128,1024] PSUM score tile so exp runs as wide ACT
instructions; the softmax denominator rides as a ones-column through the
PV matmul; per-block normalization is reciprocal_approx_fast (DVE) +
GpSimd partition broadcast + DVE multiply. The output projection is
interleaved per 512-column chunk into the ACT-bound attention window.
"""

from contextlib import ExitStack

import ml_dtypes
import numpy as np

import concourse.bass as bass
import concourse.mybir as mybir
import concourse.tile as tile
from concourse import bacc
from concourse.bass_utils import run_bass_kernel_spmd

F32 = mybir.dt.float32
BF16 = mybir.dt.bfloat16
AF = mybir.ActivationFunctionType

B, L, D = 4, 2048, 1024
H, HD = 16, 64          # global heads, head dim
HPC = 8                 # heads per core
DH = HPC * HD           # 512: per-core projected width
KT = L // 128           # 16 key tiles
NCORES = 8
ROPE_BASE = 10000.0

_cache: dict = {}


def _build(compile=True):
    if "nc" in _cache:
        return _cache["nc"]

    nc = bacc.Bacc("TRN2", target_bir_lowering=False, debug=False)

    qTd = nc.dram_tensor("qT", [D, L], BF16, kind="ExternalInput").ap()
    kTd = nc.dram_tensor("kT", [D, L], BF16, kind="ExternalInput").ap()
    vTd = nc.dram_tensor("vT", [D, L], BF16, kind="ExternalInput").ap()
    wqd = nc.dram_tensor("wq", [128, 8 * DH], BF16, kind="ExternalInput").ap()
    wkd = nc.dram_tensor("wk", [128, 8 * DH], BF16, kind="ExternalInput").ap()
    wvd = nc.dram_tensor("wv", [128, 8 * DH], BF16, kind="ExternalInput").ap()
    wod = nc.dram_tensor("wo", [128, 4 * D], BF16, kind="ExternalInput").ap()
    bqc = nc.dram_tensor("bqc", [128, 4], F32, kind="ExternalInput").ap()
    bkc = nc.dram_tensor("bkc", [128, 4], F32, kind="ExternalInput").ap()
    cosP = nc.dram_tensor("cosP", [128, L], BF16, kind="ExternalInput").ap()
    sinP = nc.dram_tensor("sinP", [128, L], BF16, kind="ExternalInput").ap()
    maskc = nc.dram_tensor("maskc", [128, 128], BF16, kind="ExternalInput").ap()
    outT = nc.dram_tensor("outT", [D, L], F32, kind="ExternalOutput").ap()
    scrd = nc.dram_tensor("scratch", [1, 512], F32, kind="ExternalOutput").ap()

    with tile.TileContext(nc) as tc, ExitStack() as ctx:
        const = ctx.enter_context(tc.tile_pool(name="const", bufs=1))
        pers = ctx.enter_context(tc.tile_pool(name="pers", bufs=1))
        px = ctx.enter_context(tc.tile_pool(name="px", bufs=3))
        ptw = ctx.enter_context(tc.tile_pool(name="ptw", bufs=4))
        psw = ctx.enter_context(tc.tile_pool(name="psw", bufs=2))
        ppt = ctx.enter_context(tc.tile_pool(name="ppt", bufs=4))
        pnm = ctx.enter_context(tc.tile_pool(name="pnm", bufs=3))
        pout = ctx.enter_context(tc.tile_pool(name="pout", bufs=4))
        pdr = ctx.enter_context(tc.tile_pool(name="pdr", bufs=3, space="DRAM"))
        pacc = ctx.enter_context(tc.tile_pool(name="pacc", bufs=2, space="PSUM"))
        psS = ctx.enter_context(tc.tile_pool(name="psS", bufs=2, space="PSUM"))
        psO = ctx.enter_context(tc.tile_pool(name="psO", bufs=2, space="PSUM"))

        # Pin the activation table to the one set that serves every ACT
        # function used here (Exp, Ln, Identity, Copy) so the table-load
        # pass never needs to thrash between per-function default tables.
        from concourse.hw_specs import get_activation_tables
        _tabs = list(get_activation_tables(nc.m.arch).items())
        _need = {AF.Exp, AF.Ln, AF.Identity, AF.Copy}
        _tid = next(i for i, (_n, s) in enumerate(_tabs) if _need <= s)
        nc.scalar.add_instruction(mybir.InstLoadActFuncSet(
            name=nc.scalar.bass.get_next_instruction_name(),
            ins=[], outs=[], act_func_set_id=_tid))

        # ---------------- constants (loads staggered in the schedule) --
        wv_t = const.tile([128, 8 * DH], BF16, tag="wv")
        wk_t = const.tile([128, 8 * DH], BF16, tag="wk")
        wq_t = const.tile([128, 8 * DH], BF16, tag="wq")
        wo_t = const.tile([128, 4 * D], BF16, tag="wo")
        cos_t = const.tile([128, L], BF16, tag="cos")
        sin_t = const.tile([128, L], BF16, tag="sin")
        mask_t = const.tile([128, 128], BF16, tag="mask")
        bq_t = const.tile([128, 4], F32, tag="bq")
        bk_t = const.tile([128, 4], F32, tag="bk")
        nc.sync.dma_start(wv_t[:], wvd[:])

        # PE warm-up: ~7us of dummy matmuls while the input DMAs stream, so
        # the HAM clock gate reaches full rate before real work arrives. The
        # result drains to a scratch output so DCE keeps it.
        wps = pacc.tile([128, 512], F32, tag="acc", name="warm")
        for it in range(16):
            nc.tensor.matmul(wps[:], wv_t[:, 0:128], wv_t[:, 0:512],
                             start=(it == 0), stop=(it == 15))
        wsb = pnm.tile([1, 512], F32, tag="rs2", name="warmsb")
        nc.scalar.copy(wsb[:], wps[0:1, :])
        nc.gpsimd.dma_start(scrd[:], wsb[:])

        kt_m = [pers.tile([128, L], BF16, tag=f"kt{m}", name=f"kt{m}")
                for m in range(4)]
        qt_m = [pers.tile([128, L], BF16, tag=f"qt{m}", name=f"qt{m}")
                for m in range(4)]
        ot_m = [pers.tile([128, L], BF16, tag=f"ot{m}", name=f"ot{m}")
                for m in range(4)]
        va = [pers.tile([128, HPC * 65], BF16, tag=f"va{t}", name=f"va{t}")
              for t in range(KT)]
        for t in range(KT):
            ones_view = va[t].rearrange("p (h x) -> p h x", x=65)[:, :, 64:65]
            nc.gpsimd.memset(ones_view, 1.0)

        # ---------------- projection emitters -------------------------
        vx_tiles = {}

        def emit_v_group(g, li_lo=0, li_hi=8, on_act=True):
            """V projection for l-tiles [li_lo, li_hi) of half g."""
            if li_lo == 0:
                vp = []
                for kk in range(8):
                    x = px.tile([128, 1024], BF16, tag=f"x{kk}",
                                name=f"vx{kk}")
                    nc.sync.dma_start(
                        x[:], vTd[kk * 128:(kk + 1) * 128,
                                  g * 1024:(g + 1) * 1024])
                    vp.append(x)
                vx_tiles[g] = vp
            vp = vx_tiles[g]
            for li in range(li_lo, li_hi):
                lt = g * 8 + li
                ps = pacc.tile([128, DH], F32, tag="acc", name=f"pv{li}")
                for kk in range(8):
                    nc.tensor.matmul(
                        ps[:],
                        vp[kk][:, li * 128:(li + 1) * 128],
                        wv_t[:, kk * DH:(kk + 1) * DH],
                        start=(kk == 0), stop=(kk == 7),
                    )
                out_view = va[lt].rearrange(
                    "p (h x) -> p h x", x=65)[:, :, 0:64]
                if on_act:
                    nc.scalar.copy(out_view, ps[:])
                else:
                    nc.vector.tensor_copy(out_view, ps[:])

        qk_x_tiles = {}

        def emit_qk_group(xT, w_t, bias_t, dst, g, mi_lo=0, mi_hi=4,
                          on_act=True, key=None):
            """Q/K projection + RoPE, m-tiles [mi_lo, mi_hi) of half g.
            mi-outer order: each m-tile is drained and RoPE'd as soon as its
            two 512-column chunks finish, so dependent attention blocks can
            start before the rest of the projection."""
            c0, c1 = g * 1024, (g + 1) * 1024
            if mi_lo == 0:
                xp = []
                for kk in range(8):
                    x = px.tile([128, 1024], BF16, tag=f"x{kk}", name=f"x{kk}")
                    nc.sync.dma_start(x[:], xT[kk * 128:(kk + 1) * 128, c0:c1])
                    xp.append(x)
                qk_x_tiles[key] = xp
            xp = qk_x_tiles[key]
            for mi in range(mi_lo, mi_hi):
                traw = ptw.tile([128, 1024], BF16, tag="traw",
                                name=f"traw{mi}")
                for njl in range(2):
                    ps = pacc.tile([128, 512], F32, tag="acc", name=f"pp{mi}")
                    for kk in range(8):
                        nc.tensor.matmul(
                            ps[:],
                            w_t[:, kk * DH + mi * 128:kk * DH + (mi + 1) * 128],
                            xp[kk][:, njl * 512:(njl + 1) * 512],
                            start=(kk == 0), stop=(kk == 7),
                        )
                    dslice = traw[:, njl * 512:(njl + 1) * 512]
                    if on_act:
                        nc.scalar.activation(dslice, ps[:], AF.Identity,
                                             bias=bias_t[:, mi:mi + 1])
                    else:
                        nc.vector.tensor_scalar_add(dslice, ps[:],
                                                    bias_t[:, mi:mi + 1])
                sw = psw.tile([128, 1024], BF16, tag="sw", name=f"sw{mi}")
                for blk in range(4):
                    srcb = blk ^ 1
                    nc.gpsimd.dma_start(
                        sw[blk * 32:(blk + 1) * 32, :],
                        traw[srcb * 32:(srcb + 1) * 32, :])
                nc.vector.tensor_mul(sw[:], sw[:], sin_t[:, c0:c1])
                nc.vector.tensor_mul(traw[:], traw[:], cos_t[:, c0:c1])
                nc.vector.tensor_add(dst[mi][:, c0:c1], traw[:], sw[:])

        # ---------------- attention + output emitters -----------------
        def emit_b_block(b, h_lo=0, h_hi=HPC):
            q0 = b * 512
            for h in range(h_lo, h_hi):
                mi, pb = h // 2, (h % 2) * 64
                o_ps = psO.tile([128, 512], F32, tag="o", name=f"o{h}")
                nkt = 4 * (b + 1)
                for p in range(nkt // 2):
                    kt0, kt1 = 2 * p, 2 * p + 1
                    s_ps = psS.tile([128, 1024], F32, tag="s", name=f"s{p}")
                    pt = ppt.tile([128, 1024], BF16, tag="pt", name=f"pt{p}")
                    subs = []
                    for i, kt in enumerate((kt0, kt1)):
                        sub = max(0, kt * 128 - q0)
                        subs.append(sub)
                        nc.tensor.matmul(
                            s_ps[:, i * 512 + sub:(i + 1) * 512],
                            kt_m[mi][pb:pb + 64, kt * 128:(kt + 1) * 128],
                            qt_m[mi][pb:pb + 64, q0 + sub:q0 + 512],
                            start=True, stop=True,
                        )
                    if subs[1] == 0:
                        nc.scalar.activation(pt[:, 0:1024], s_ps[:, 0:1024],
                                             AF.Exp, scale=0.125)
                    else:
                        # diagonal pair: exp only the computed ranges
                        nc.scalar.activation(
                            pt[:, subs[0]:512], s_ps[:, subs[0]:512],
                            AF.Exp, scale=0.125)
                        nc.scalar.activation(
                            pt[:, 512 + subs[1]:1024],
                            s_ps[:, 512 + subs[1]:1024],
                            AF.Exp, scale=0.125)
                    for i, kt in enumerate((kt0, kt1)):
                        sub = subs[i]
                        diag = kt >= 4 * b
                        lhs = va[kt][:, h * 65:h * 65 + 65]
                        if diag:
                            nc.vector.tensor_mul(
                                pt[:, i * 512 + sub:i * 512 + sub + 128],
                                pt[:, i * 512 + sub:i * 512 + sub + 128],
                                mask_t[:])
                        nc.tensor.matmul(
                            o_ps[0:65, sub:512], lhs,
                            pt[:, i * 512 + sub:(i + 1) * 512],
                            start=(kt == 0), stop=(kt == nkt - 1),
                            skip_group_check=True,
                        )
                # normalize block: ot = o * (1/colsum). The reciprocal is
                # exp(-ln(colsum)) on the ACT engine (Ln and Exp share one
                # activation table), avoiding both the DVE's serial per-lane
                # reciprocal and any cross-partition DMA bounce; o and the
                # colsum row are copied out of PSUM first so the accumulator
                # bank recycles immediately.
                o_sb = pnm.tile([65, 512], F32, tag="osb", name=f"on{h}")
                nc.vector.tensor_copy(o_sb[:], o_ps[0:65, :])
                rs2 = pnm.tile([1, 512], F32, tag="rs2", name=f"rs2{h}")
                if b == 3 and h < 7:
                    # ACT is exp-saturated in the last block; bounce the
                    # reciprocal through DRAM on the (idle) SP queue instead
                    d1 = pdr.tile([1, 512], F32, tag="d1", name=f"d1{h}")
                    nc.sync.dma_start(d1[:], o_sb[64:65, :])
                    r4 = pnm.tile([128, 4], F32, tag="r4", name=f"r4{h}")
                    nc.sync.dma_start(
                        r4[:], d1.rearrange("o (p x) -> (o p) x", p=128))
                    rr = pnm.tile([128, 4], F32, tag="rr", name=f"rr{h}")
                    nc.vector.reciprocal(rr[:], r4[:])
                    d2 = pdr.tile([1, 512], F32, tag="d2", name=f"d2{h}")
                    nc.sync.dma_start(
                        d2[:].rearrange("o (p x) -> (o p) x", p=128), rr[:])
                    nc.sync.dma_start(rs2[:], d2[:])
                else:
                    lncs = pnm.tile([1, 512], F32, tag="lncs", name=f"ln{h}")
                    nc.scalar.activation(lncs[:], o_sb[64:65, :], AF.Ln)
                    nc.scalar.activation(rs2[:], lncs[:], AF.Exp, scale=-1.0)
                rcb = pnm.tile([64, 512], F32, tag="rcb", name=f"rcb{h}")
                nc.gpsimd.partition_broadcast(rcb[:], rs2[:], channels=64)
                nc.vector.tensor_mul(
                    ot_m[mi][pb:pb + 64, q0:q0 + 512], o_sb[0:64, :], rcb[:])

        def emit_c(nj):
            for mo in range(8):
                ps = pacc.tile([128, 512], F32, tag="acc", name=f"c{mo}")
                for kti in range(4):
                    nc.tensor.matmul(
                        ps[:],
                        wo_t[:, kti * D + mo * 128:kti * D + (mo + 1) * 128],
                        ot_m[kti][:, nj * 512:(nj + 1) * 512],
                        start=(kti == 0), stop=(kti == 3),
                    )
                osb = pout.tile([128, 512], F32, tag="osb", name=f"osb{mo}")
                nc.vector.tensor_copy(osb[:], ps[:])
                # split over partition halves -> two DMA queues in parallel
                for ph in range(2):
                    nc.gpsimd.dma_start(
                        outT[mo * 128 + ph * 64:mo * 128 + (ph + 1) * 64,
                             nj * 512:(nj + 1) * 512],
                        osb[ph * 64:(ph + 1) * 64, :])

        # ---------------- schedule ------------------------------------
        # Fine-grained interleave: attention head-pairs start as soon as
        # their K/Q m-tile and V l-tiles exist; second-half projections
        # backfill the PE while the ACT engine chews block exps.
        emit_v_group(0, 0, 4, on_act=True)
        nc.sync.dma_start(wk_t[:], wkd[:])
        nc.sync.dma_start(bk_t[:], bkc[:])
        nc.sync.dma_start(bq_t[:], bqc[:])
        nc.sync.dma_start(cos_t[:], cosP[:])
        nc.sync.dma_start(sin_t[:], sinP[:])
        nc.sync.dma_start(mask_t[:], maskc[:])
        emit_qk_group(kTd, wk_t, bk_t, kt_m, 0, 0, 1, on_act=True, key="k0")
        nc.sync.dma_start(wq_t[:], wqd[:])
        emit_qk_group(qTd, wq_t, bq_t, qt_m, 0, 0, 1, on_act=True, key="q0")
        nc.sync.dma_start(wo_t[:], wod[:])
        emit_b_block(0, 0, 2)
        emit_v_group(0, 4, 8, on_act=True)
        emit_qk_group(kTd, wk_t, bk_t, kt_m, 0, 1, 2, on_act=True, key="k0")
        emit_qk_group(qTd, wq_t, bq_t, qt_m, 0, 1, 2, on_act=True, key="q0")
        emit_b_block(0, 2, 4)
        emit_qk_group(kTd, wk_t, bk_t, kt_m, 0, 2, 3, on_act=True, key="k0")
        emit_qk_group(qTd, wq_t, bq_t, qt_m, 0, 2, 3, on_act=True, key="q0")
        emit_b_block(0, 4, 6)
        emit_qk_group(kTd, wk_t, bk_t, kt_m, 0, 3, 4, on_act=True, key="k0")
        emit_qk_group(qTd, wq_t, bq_t, qt_m, 0, 3, 4, on_act=True, key="q0")
        emit_b_block(0, 6, 8)
        emit_b_block(1, 0, 2)
        emit_v_group(1, 0, 4, on_act=False)
        emit_b_block(1, 2, 4)
        emit_v_group(1, 4, 8, on_act=False)
        emit_b_block(1, 4, 6)
        emit_qk_group(kTd, wk_t, bk_t, kt_m, 1, 0, 2, on_act=False, key="k1")
        emit_b_block(1, 6, 8)
        emit_qk_group(kTd, wk_t, bk_t, kt_m, 1, 2, 4, on_act=False, key="k1")
        emit_qk_group(qTd, wq_t, bq_t, qt_m, 1, 0, 4, on_act=False, key="q1")
        emit_b_block(2)
        emit_c(0)
        emit_c(1)
        emit_b_block(3, 0, 4)
        emit_c(2)
        emit_b_block(3, 4, 8)
        emit_c(3)

    if compile:
        nc.compile()
        _cache["nc"] = nc
    return nc


def _prep(q, k, v, Wq, bq, Wk, bk, Wv, bv, Wo, bo):
    """Build the 8 per-core input maps (host-side shard + layout prep)."""
    bf16 = ml_dtypes.bfloat16
    # de-interleave permutation within each head: evens then odds
    perm = np.concatenate([np.arange(0, HD, 2), np.arange(1, HD, 2)])

    # RoPE tables in de-interleaved layout, tiled x2 over partitions
    inv_freq = 1.0 / (ROPE_BASE ** (np.arange(0, HD // 2, dtype=np.float64)
                                    * 2.0 / HD))
    t = np.arange(L, dtype=np.float64)
    freqs = inv_freq[:, None] * t[None, :]            # [32, L]
    cos64 = np.cos(np.concatenate([freqs, freqs], axis=0))   # [64, L]
    sin64 = np.sin(np.concatenate([freqs, freqs], axis=0))
    sin64[:32] *= -1.0
    cosP = np.tile(cos64, (2, 1)).astype(bf16)        # [128, L]
    sinP = np.tile(sin64, (2, 1)).astype(bf16)

    # causal mask in S^T space: keep k <= q
    kk, qq = np.meshgrid(np.arange(128), np.arange(128), indexing="ij")
    mask = (kk <= qq).astype(bf16)

    qTb = [np.ascontiguousarray(q[b_i].T.astype(bf16)) for b_i in range(B)]
    kTb = [np.ascontiguousarray(k[b_i].T.astype(bf16)) for b_i in range(B)]
    vTb = [np.ascontiguousarray(v[b_i].T.astype(bf16)) for b_i in range(B)]

    def wtile(WT, nk):
        # [nk*128, F] -> [128, nk*F] with kk-slices side by side
        F = WT.shape[1]
        return np.ascontiguousarray(
            WT.reshape(nk, 128, F).transpose(1, 0, 2).reshape(128, nk * F)
            .astype(bf16))

    in_maps = []
    for c in range(NCORES):
        b_i, hg = c // 2, c % 2
        rows = hg * DH + (np.arange(DH).reshape(HPC, HD)[:, perm]).reshape(-1)
        in_maps.append({
            "qT": qTb[b_i],
            "kT": kTb[b_i],
            "vT": vTb[b_i],
            "wq": wtile(Wq[rows, :].T, 8),
            "wk": wtile(Wk[rows, :].T, 8),
            "wv": wtile(Wv[hg * DH:(hg + 1) * DH, :].T, 8),
            "wo": wtile(Wo[:, hg * DH:(hg + 1) * DH].T, 4),
            "bqc": np.ascontiguousarray(bq[rows].reshape(4, 128).T),
            "bkc": np.ascontiguousarray(bk[rows].reshape(4, 128).T),
            "cosP": cosP,
            "sinP": sinP,
            "maskc": mask,
        })
    return in_maps


def _assemble(results, bo, Wo, bv):
    bo_eff = (bo + Wo.astype(np.float64) @ bv.astype(np.float64)).astype(
        np.float32)
    out = np.empty((B, L, D), dtype=np.float32)
    for b_i in range(B):
        acc = results[2 * b_i]["outT"] + results[2 * b_i + 1]["outT"]
        out[b_i] = acc.T + bo_eff[None, :]
    return out


def kernel(q, k, v, Wq, bq, Wk, bk, Wv, bv, Wo, bo):
    q = np.asarray(q, dtype=np.float32)
    k = np.asarray(k, dtype=np.float32)
    v = np.asarray(v, dtype=np.float32)
    Wq = np.asarray(Wq, dtype=np.float32)
    Wk = np.asarray(Wk, dtype=np.float32)
    Wv = np.asarray(Wv, dtype=np.float32)
    Wo = np.asarray(Wo, dtype=np.float32)
    bq = np.asarray(bq, dtype=np.float32)
    bk = np.asarray(bk, dtype=np.float32)
    bv = np.asarray(bv, dtype=np.float32)
    bo = np.asarray(bo, dtype=np.float32)

    nc = _build()
    in_maps = _prep(q, k, v, Wq, bq, Wk, bk, Wv, bv, Wo, bo)
    res = run_bass_kernel_spmd(nc, in_maps, core_ids=list(range(NCORES)))
    return _assemble(res.results, bo, Wo, bv)
